# revision 3
# baseline (speedup 1.0000x reference)
"""Trainium2 Bass kernel for nn_Attention_23613730194049.

Reference computation (per batch element b, B=8, N=2048, D=512):
    q = X @ WQ_w.T + WQ_b
    k = X @ WK_w.T + WK_b
    v = X @ WV_w.T + WV_b
    scores = (q @ k.T) / sqrt(D)
    attn = softmax(scores, axis=-1) + intensity      # post-softmax additive bias
    out = (attn @ v) @ out_w.T + out_b

Sharding: data-parallel over batch. Each of the 8 NeuronCores gets one batch
element (X[b], intensity[b]) plus replicated weights; no collectives.

Per-core layout strategy:
  - X is transposed on the PE (identity-matmul transpose) into X^T [d, n] so
    the d-contraction of the projections has d on partitions.
  - Weights [out,in] are PE-transposed to [in,out].
  - q^T, k^T are produced in [e, n] layout; v in [n, e] layout.
  - scores row-blocks S_i = [128, 2048] are built in PSUM ([i-part, j-free]),
    softmax runs along the free dim: one ACT pass does exp(scale*S) and the
    row-sum (accum_out); normalize + add-intensity is a single fused DVE
    scalar_tensor_tensor op.
  - attn row-blocks are PE-transposed to feed attn @ v; the result is
    transposed again for the final projection so the output lands in natural
    [n, e] layout for the store.

Matmul operands are bitcast to float32r (full-rate PE mode, fp32 storage).
"""

import math

import numpy as np

B = 8
N = 2048
D = 512
P = 128
NT = N // P  # 16 row tiles
DT = D // P  # 4 feature tiles
CH = 512  # moving-operand chunk (one PSUM bank of fp32)
NCH = N // CH  # 4
SCALE = 1.0 / math.sqrt(D)

# "f32r" = float32r matmul operands (fast PE mode), "f32" = plain fp32.
MM_MODE = "f32r"

_CACHE = {}


def _emit(nc, tc, aps):
    import concourse.bass as bass
    from concourse import mybir
    from concourse.masks import make_identity
    from contextlib import ExitStack

    f32 = mybir.dt.float32
    f32r = mybir.dt.float32r
    Act = mybir.ActivationFunctionType
    Alu = mybir.AluOpType

    X, INT, WQ, WQb, WK, WKb, WV, WVb, OW, OWb, OUT = aps

    # Matmul-operand tiles are allocated as float32r: the producing engine op
    # (copy / activation / scalar_tensor_tensor) rounds into the PE's fast
    # fp32 mode, which the BIR verifier requires for FP32r matmult inputs.
    mdt = f32r if MM_MODE == "f32r" else f32

    with ExitStack() as ctx:
        persist = ctx.enter_context(tc.tile_pool(name="persist", bufs=1))
        consts = ctx.enter_context(tc.tile_pool(name="consts", bufs=1))
        ps_pool = ctx.enter_context(tc.tile_pool(name="ps", bufs=3, space="PSUM"))
        tp_pool = ctx.enter_context(tc.tile_pool(name="tp", bufs=2, space="PSUM"))
        av_pool = ctx.enter_context(tc.tile_pool(name="av", bufs=2, space="PSUM"))

        ident = consts.tile([P, P], f32, name="ident", tag="ident")
        make_identity(nc, ident[:])

        # q/k biases as [128, 4] (column t = b[t*128:(t+1)*128]) for per-partition
        # ACT bias in the [e, n] layouts.
        qb = consts.tile([P, DT], f32, name="qb", tag="qb")
        nc.sync.dma_start(out=qb[:], in_=WQb.rearrange("(t p) -> p t", p=P))
        kb = consts.tile([P, DT], f32, name="kb", tag="kb")
        nc.sync.dma_start(out=kb[:], in_=WKb.rearrange("(t p) -> p t", p=P))

        # v / out biases broadcast across partitions (bias varies along free dim).
        vb_bc = consts.tile([P, D], f32, name="vb_bc", tag="vb_bc")
        nc.gpsimd.dma_start(
            out=vb_bc[:],
            in_=bass.AP(tensor=WVb.tensor, offset=WVb.offset, ap=[[0, P], [1, D]]),
        )
        ob_bc = consts.tile([P, D], f32, name="ob_bc", tag="ob_bc")
        nc.gpsimd.dma_start(
            out=ob_bc[:],
            in_=bass.AP(tensor=OWb.tensor, offset=OWb.offset, ap=[[0, P], [1, D]]),
        )

        # Persistent activations for the attention phase.
        qT = [persist.tile([P, N], mdt, name=f"qT{d}", tag=f"qT{d}") for d in range(DT)]
        kT = [persist.tile([P, N], mdt, name=f"kT{d}", tag=f"kT{d}") for d in range(DT)]
        vt = [persist.tile([P, D], mdt, name=f"v{j}", tag=f"v{j}") for j in range(NT)]
        owT = [persist.tile([P, D], mdt, name=f"owT{d}", tag=f"owT{d}") for d in range(DT)]

        # ---------------- Phase 0 + 1: transposes and projections ----------------
        with tc.tile_pool(name="ph01", bufs=3) as ph01:
            xT = [
                ph01.tile([P, N], mdt, name=f"xT{d}", tag=f"xT{d}", bufs=1)
                for d in range(DT)
            ]
            wqT = [
                ph01.tile([P, D], mdt, name=f"wqT{d}", tag=f"wqT{d}", bufs=1)
                for d in range(DT)
            ]
            wkT = [
                ph01.tile([P, D], mdt, name=f"wkT{d}", tag=f"wkT{d}", bufs=1)
                for d in range(DT)
            ]
            wvT = [
                ph01.tile([P, D], mdt, name=f"wvT{d}", tag=f"wvT{d}", bufs=1)
                for d in range(DT)
            ]

            # Transpose the four weight matrices: [out, in] -> [in, out].
            for wap, wdst in ((WQ, wqT), (WK, wkT), (WV, wvT), (OW, owT)):
                for et in range(DT):
                    wnat = ph01.tile([P, D], f32, name="wnat", tag="wnat")
                    nc.sync.dma_start(out=wnat[:], in_=wap[et * P : (et + 1) * P, :])
                    for d in range(DT):
                        pt = tp_pool.tile([P, P], f32, name="tp", tag="tp")
                        nc.tensor.transpose(pt[:], wnat[:, d * P : (d + 1) * P], ident[:])
                        cp = nc.scalar.copy if (et + d) % 2 else nc.vector.tensor_copy
                        cp(wdst[d][:, et * P : (et + 1) * P], pt[:])

            # Transpose X: [n, d] -> [d, n].
            for nt in range(NT):
                xnat = ph01.tile([P, D], f32, name="xnat", tag="xnat")
                nc.sync.dma_start(out=xnat[:], in_=X[nt * P : (nt + 1) * P, :])
                for d in range(DT):
                    pt = tp_pool.tile([P, P], f32, name="tp", tag="tp")
                    nc.tensor.transpose(pt[:], xnat[:, d * P : (d + 1) * P], ident[:])
                    cp = nc.scalar.copy if (nt + d) % 2 else nc.vector.tensor_copy
                    cp(xT[d][:, nt * P : (nt + 1) * P], pt[:])

            # q^T = WQ @ X^T + b (and k^T): [e, n] layout, bias per-partition.
            for wT, bcol, dstT in ((wqT, qb, qT), (wkT, kb, kT)):
                for et in range(DT):
                    for nch in range(NCH):
                        ps = ps_pool.tile([P, CH], f32, name="ps", tag="ps")
                        for d in range(DT):
                            nc.tensor.matmul(
                                ps[:],
                                wT[d][:, et * P : (et + 1) * P],
                                xT[d][:, nch * CH : (nch + 1) * CH],
                                start=(d == 0),
                                stop=(d == DT - 1),
                            )
                        nc.scalar.activation(
                            dstT[et][:, nch * CH : (nch + 1) * CH],
                            ps[:],
                            Act.Identity,
                            bias=bcol[:, et : et + 1],
                            scale=1.0,
                        )

            # v = X @ WV^T + b: [n, e] layout, bias broadcast along free dim.
            for nt in range(NT):
                ps = ps_pool.tile([P, D], f32, name="ps", tag="ps")
                for d in range(DT):
                    nc.tensor.matmul(
                        ps[:],
                        xT[d][:, nt * P : (nt + 1) * P],
                        wvT[d][:],
                        start=(d == 0),
                        stop=(d == DT - 1),
                    )
                nc.vector.scalar_tensor_tensor(
                    out=vt[nt][:],
                    in0=ps[:],
                    scalar=0.0,
                    in1=vb_bc[:],
                    op0=Alu.bypass,
                    op1=Alu.add,
                )

        # ---------------- Phase 2: attention, one 128-row block at a time --------
        e_pool = ctx.enter_context(tc.tile_pool(name="e", bufs=2))
        int_pool = ctx.enter_context(tc.tile_pool(name="intp", bufs=2))
        at_pool = ctx.enter_context(tc.tile_pool(name="at", bufs=2))
        sm_pool = ctx.enter_context(tc.tile_pool(name="sm", bufs=2))
        oa_pool = ctx.enter_context(tc.tile_pool(name="oa", bufs=2))
        of_pool = ctx.enter_context(tc.tile_pool(name="of", bufs=2))

        for it in range(NT):
            int_t = int_pool.tile([P, N], f32, name="int_t", tag="int")
            nc.sync.dma_start(out=int_t[:], in_=INT[it * P : (it + 1) * P, :])

            # S row-block -> exp + row-sum, chunk by chunk.
            E = e_pool.tile([P, N], f32, name="E", tag="E")
            racc = sm_pool.tile([P, NCH], f32, name="racc", tag="racc")
            for jc in range(NCH):
                ps = ps_pool.tile([P, CH], f32, name="ps", tag="ps")
                for et in range(DT):
                    nc.tensor.matmul(
                        ps[:],
                        qT[et][:, it * P : (it + 1) * P],
                        kT[et][:, jc * CH : (jc + 1) * CH],
                        start=(et == 0),
                        stop=(et == DT - 1),
                    )
                nc.scalar.activation(
                    E[:, jc * CH : (jc + 1) * CH],
                    ps[:],
                    Act.Exp,
                    bias=0.0,
                    scale=SCALE,
                    accum_out=racc[:, jc : jc + 1],
                )

            r = sm_pool.tile([P, 1], f32, name="r", tag="r")
            nc.vector.reduce_sum(out=r[:], in_=racc[:], axis=mybir.AxisListType.X)
            rinv = sm_pool.tile([P, 1], f32, name="rinv", tag="rinv")
            nc.vector.reciprocal(rinv[:], r[:])

            # attn = E * (1/rowsum) + intensity, fused, in place over E.
            nc.vector.scalar_tensor_tensor(
                out=E[:],
                in0=E[:],
                scalar=rinv[:],
                in1=int_t[:],
                op0=Alu.mult,
                op1=Alu.add,
            )

            # Transpose the attn row-block: 16 PE transposes -> [j, i] tiles.
            atts = []
            for jt in range(NT):
                pt = tp_pool.tile([P, P], f32, name="tp", tag="tp")
                nc.tensor.transpose(pt[:], E[:, jt * P : (jt + 1) * P], ident[:])
                at = at_pool.tile([P, P], mdt, name=f"at{jt}", tag=f"at{jt}")
                cp = nc.scalar.copy if jt % 2 else nc.vector.tensor_copy
                cp(at[:], pt[:])
                atts.append(at)

            # out_attn[i, e] = sum_j attn^T[j, i].T @ v[j, e]
            ps2 = av_pool.tile([P, D], f32, name="ps2", tag="av")
            for jt in range(NT):
                nc.tensor.matmul(
                    ps2[:],
                    atts[jt][:],
                    vt[jt][:],
                    start=(jt == 0),
                    stop=(jt == NT - 1),
                )
            oa = oa_pool.tile([P, D], f32, name="oa", tag="oa")
            nc.scalar.copy(oa[:], ps2[:])

            # Transpose out_attn block for the final projection.
            oats = []
            for et in range(DT):
                pt = tp_pool.tile([P, P], f32, name="tp", tag="tp")
                nc.tensor.transpose(pt[:], oa[:, et * P : (et + 1) * P], ident[:])
                oat = oa_pool.tile([P, P], mdt, name=f"oat{et}", tag=f"oat{et}")
                nc.vector.tensor_copy(oat[:], pt[:])
                oats.append(oat)

            # out = out_attn @ OW^T + b, in natural [n, e] layout.
            psf = ps_pool.tile([P, D], f32, name="psf", tag="ps")
            for et in range(DT):
                nc.tensor.matmul(
                    psf[:],
                    oats[et][:],
                    owT[et][:],
                    start=(et == 0),
                    stop=(et == DT - 1),
                )
            of = of_pool.tile([P, D], f32, name="of", tag="of")
            nc.vector.scalar_tensor_tensor(
                out=of[:],
                in0=psf[:],
                scalar=0.0,
                in1=ob_bc[:],
                op0=Alu.bypass,
                op1=Alu.add,
            )
            nc.sync.dma_start(out=OUT[it * P : (it + 1) * P, :], in_=of[:])


def build():
    import concourse.tile as tile
    from concourse import bacc, mybir

    f32 = mybir.dt.float32

    nc = bacc.Bacc("TRN2", target_bir_lowering=False, debug=False, num_devices=B)

    X = nc.dram_tensor("X", [N, D], f32, kind="ExternalInput").ap()
    INT = nc.dram_tensor("intensity", [N, N], f32, kind="ExternalInput").ap()
    WQ = nc.dram_tensor("WQ_w", [D, D], f32, kind="ExternalInput").ap()
    WQb = nc.dram_tensor("WQ_b", [D], f32, kind="ExternalInput").ap()
    WK = nc.dram_tensor("WK_w", [D, D], f32, kind="ExternalInput").ap()
    WKb = nc.dram_tensor("WK_b", [D], f32, kind="ExternalInput").ap()
    WV = nc.dram_tensor("WV_w", [D, D], f32, kind="ExternalInput").ap()
    WVb = nc.dram_tensor("WV_b", [D], f32, kind="ExternalInput").ap()
    OW = nc.dram_tensor("out_w", [D, D], f32, kind="ExternalInput").ap()
    OWb = nc.dram_tensor("out_b", [D], f32, kind="ExternalInput").ap()
    OUT = nc.dram_tensor("out", [N, D], f32, kind="ExternalOutput").ap()

    aps = (X, INT, WQ, WQb, WK, WKb, WV, WVb, OW, OWb, OUT)
    with tile.TileContext(nc) as tc:
        _emit(nc, tc, aps)
    nc.compile()
    return nc


def get_nc():
    if "nc" not in _CACHE:
        _CACHE["nc"] = build()
    return _CACHE["nc"]


def make_in_maps(**inputs):
    X = np.asarray(inputs["X"], dtype=np.float32)
    INT = np.asarray(inputs["intensity"], dtype=np.float32)
    shared = {
        name: np.asarray(inputs[name], dtype=np.float32)
        for name in ("WQ_w", "WQ_b", "WK_w", "WK_b", "WV_w", "WV_b", "out_w", "out_b")
    }
    return [{"X": X[b], "intensity": INT[b], **shared} for b in range(B)]


def kernel(**inputs):
    from concourse.bass_utils import run_bass_kernel_spmd

    nc = get_nc()
    in_maps = make_in_maps(**inputs)
    res = run_bass_kernel_spmd(nc, in_maps, core_ids=list(range(B)))
    return np.stack([res.results[b]["out"] for b in range(B)]).astype(np.float32)


# revision 6
# speedup vs baseline: 55.3622x; 55.3622x over previous
"""Trainium2 Bass kernel for nn_Attention_23613730194049.

Reference computation (per batch element b, B=8, N=2048, D=512):
    q = X @ WQ_w.T + WQ_b
    k = X @ WK_w.T + WK_b
    v = X @ WV_w.T + WV_b
    scores = (q @ k.T) / sqrt(D)
    attn = softmax(scores, axis=-1) + intensity      # post-softmax additive bias
    out = (attn @ v) @ out_w.T + out_b

Sharding: data-parallel over batch. Each of the 8 NeuronCores gets one batch
element (X[b], intensity[b]) plus replicated weights; no collectives.

Per-core layout strategy:
  - X is transposed on the PE (identity-matmul transpose) into X^T [d, n] so
    the d-contraction of the projections has d on partitions.
  - Weights [out,in] are PE-transposed to [in,out].
  - q^T, k^T are produced in [e, n] layout; v in [n, e] layout.
  - scores row-blocks S_i = [128, 2048] are built in PSUM ([i-part, j-free]),
    softmax runs along the free dim: one ACT pass does exp(scale*S) and the
    row-sum (accum_out); normalize + add-intensity is a single fused DVE
    scalar_tensor_tensor op.
  - attn row-blocks are PE-transposed to feed attn @ v; the result is
    transposed again for the final projection so the output lands in natural
    [n, e] layout for the store.

Matmul-operand tiles are stored as float32r (full-rate PE mode, fp32 bytes).
"""

import math

import numpy as np

B = 8
N = 2048
D = 512
P = 128
NT = N // P  # 16 row tiles
DT = D // P  # 4 feature tiles
CH = 512  # moving-operand chunk (one PSUM bank of fp32)
NCH = N // CH  # 4
SCALE = 1.0 / math.sqrt(D)

# "f32r" = float32r matmul operands (fast PE mode), "f32" = plain fp32.
MM_MODE = "f32r"

_CACHE = {}


def _emit(nc, tc, aps):
    import concourse.bass as bass
    from concourse import mybir
    from concourse.masks import make_identity
    from contextlib import ExitStack

    f32 = mybir.dt.float32
    f32r = mybir.dt.float32r
    Act = mybir.ActivationFunctionType
    Alu = mybir.AluOpType

    X, INT, WQ, WQb, WK, WKb, WV, WVb, OW, OWb, OUT = aps

    # Matmul-operand tiles are allocated as float32r: the producing engine op
    # (copy / activation / scalar_tensor_tensor) rounds into the PE's fast
    # fp32 mode, which the BIR verifier requires for FP32r matmult inputs.
    mdt = f32r if MM_MODE == "f32r" else f32

    with ExitStack() as ctx:
        persist = ctx.enter_context(tc.tile_pool(name="persist", bufs=1))
        consts = ctx.enter_context(tc.tile_pool(name="consts", bufs=1))
        ps_pool = ctx.enter_context(tc.tile_pool(name="ps", bufs=3, space="PSUM"))
        tp_pool = ctx.enter_context(tc.tile_pool(name="tp", bufs=2, space="PSUM"))
        av_pool = ctx.enter_context(tc.tile_pool(name="av", bufs=2, space="PSUM"))

        ident = consts.tile([P, P], f32, name="ident", tag="ident")
        make_identity(nc, ident[:])

        # q/k biases as [128, 4] (column t = b[t*128:(t+1)*128]) for per-partition
        # ACT bias in the [e, n] layouts.
        qb = consts.tile([P, DT], f32, name="qb", tag="qb")
        nc.sync.dma_start(out=qb[:], in_=WQb.rearrange("(t p) -> p t", p=P))
        kb = consts.tile([P, DT], f32, name="kb", tag="kb")
        nc.sync.dma_start(out=kb[:], in_=WKb.rearrange("(t p) -> p t", p=P))

        # v / out biases broadcast across partitions (bias varies along free dim).
        vb_bc = consts.tile([P, D], f32, name="vb_bc", tag="vb_bc")
        nc.gpsimd.dma_start(
            out=vb_bc[:],
            in_=bass.AP(tensor=WVb.tensor, offset=WVb.offset, ap=[[0, P], [1, D]]),
        )
        ob_bc = consts.tile([P, D], f32, name="ob_bc", tag="ob_bc")
        nc.gpsimd.dma_start(
            out=ob_bc[:],
            in_=bass.AP(tensor=OWb.tensor, offset=OWb.offset, ap=[[0, P], [1, D]]),
        )

        # Persistent activations for the attention phase.
        qT = [persist.tile([P, N], mdt, name=f"qT{d}", tag=f"qT{d}") for d in range(DT)]
        kT = [persist.tile([P, N], mdt, name=f"kT{d}", tag=f"kT{d}") for d in range(DT)]
        vt = [persist.tile([P, D], mdt, name=f"v{j}", tag=f"v{j}") for j in range(NT)]
        owT = [persist.tile([P, D], mdt, name=f"owT{d}", tag=f"owT{d}") for d in range(DT)]

        # ---------------- Phase 0 + 1: transposes and projections ----------------
        # Interleaved so the PE never sits idle waiting on PSUM->SBUF copies:
        # weights first (DMAs start early), then X-transposes and projections
        # alternating per 4-tile chunk.
        with tc.tile_pool(name="ph01", bufs=3) as ph01:
            xT = [
                ph01.tile([P, N], mdt, name=f"xT{d}", tag=f"xT{d}", bufs=1)
                for d in range(DT)
            ]
            wqT = [
                ph01.tile([P, D], mdt, name=f"wqT{d}", tag=f"wqT{d}", bufs=1)
                for d in range(DT)
            ]
            wkT = [
                ph01.tile([P, D], mdt, name=f"wkT{d}", tag=f"wkT{d}", bufs=1)
                for d in range(DT)
            ]
            wvT = [
                ph01.tile([P, D], mdt, name=f"wvT{d}", tag=f"wvT{d}", bufs=1)
                for d in range(DT)
            ]

            # Transpose the four weight matrices: [out, in] -> [in, out].
            # 4 transposes share one PSUM bank; one [128, 512] copy per bank.
            for wap, wdst in ((WQ, wqT), (WK, wkT), (WV, wvT), (OW, owT)):
                for et in range(DT):
                    wnat = ph01.tile([P, D], f32, name="wnat", tag="wnat")
                    nc.sync.dma_start(out=wnat[:], in_=wap[et * P : (et + 1) * P, :])
                    pt = tp_pool.tile([P, D], f32, name="tp4", tag="tp4")
                    for d in range(DT):
                        nc.tensor.transpose(
                            pt[:, d * P : (d + 1) * P],
                            wnat[:, d * P : (d + 1) * P],
                            ident[:],
                        )
                    # Scatter columns of the bank into the 4 destination tiles.
                    for d in range(DT):
                        cp = nc.scalar.copy if (et + d) % 2 else nc.vector.tensor_copy
                        cp(wdst[d][:, et * P : (et + 1) * P], pt[:, d * P : (d + 1) * P])

            # X-transpose + projections, chunk by chunk (4 n-tiles per chunk).
            for c in range(NCH):
                for nt in range(4 * c, 4 * c + 4):
                    xnat = ph01.tile([P, D], f32, name="xnat", tag="xnat")
                    nc.sync.dma_start(out=xnat[:], in_=X[nt * P : (nt + 1) * P, :])
                    pt = tp_pool.tile([P, D], f32, name="tp4", tag="tp4")
                    for d in range(DT):
                        nc.tensor.transpose(
                            pt[:, d * P : (d + 1) * P],
                            xnat[:, d * P : (d + 1) * P],
                            ident[:],
                        )
                    for d in range(DT):
                        cp = nc.scalar.copy if (nt + d) % 2 else nc.vector.tensor_copy
                        cp(xT[d][:, nt * P : (nt + 1) * P], pt[:, d * P : (d + 1) * P])

                # q^T / k^T for this n-chunk: [e, n] layout, bias per-partition.
                for wT, bcol, dstT in ((wqT, qb, qT), (wkT, kb, kT)):
                    for et in range(DT):
                        ps = ps_pool.tile([P, CH], f32, name="ps", tag="ps")
                        for d in range(DT):
                            nc.tensor.matmul(
                                ps[:],
                                wT[d][:, et * P : (et + 1) * P],
                                xT[d][:, c * CH : (c + 1) * CH],
                                start=(d == 0),
                                stop=(d == DT - 1),
                            )
                        nc.scalar.activation(
                            dstT[et][:, c * CH : (c + 1) * CH],
                            ps[:],
                            Act.Identity,
                            bias=bcol[:, et : et + 1],
                            scale=1.0,
                        )

                # v for this chunk's n-tiles: [n, e] layout, broadcast bias.
                for nt in range(4 * c, 4 * c + 4):
                    ps = ps_pool.tile([P, D], f32, name="ps", tag="ps")
                    for d in range(DT):
                        nc.tensor.matmul(
                            ps[:],
                            xT[d][:, nt * P : (nt + 1) * P],
                            wvT[d][:],
                            start=(d == 0),
                            stop=(d == DT - 1),
                        )
                    nc.vector.scalar_tensor_tensor(
                        out=vt[nt][:],
                        in0=ps[:],
                        scalar=0.0,
                        in1=vb_bc[:],
                        op0=Alu.bypass,
                        op1=Alu.add,
                    )

        # ---------------- Phase 2: attention, software-pipelined -----------------
        # Stage A(it): intensity DMA, scores matmuls, exp+rowsum (ACT).
        # Stage B(it): normalize+add-intensity, transposes, attn@v, final proj.
        # B(it-1) is emitted after A(it) so the in-order PE queue always has
        # score matmuls to chew on while the softmax chain of the previous
        # block finishes on ACT/DVE.
        e_pool = ctx.enter_context(tc.tile_pool(name="e", bufs=2))
        int_pool = ctx.enter_context(tc.tile_pool(name="intp", bufs=2))
        at_pool = ctx.enter_context(tc.tile_pool(name="at", bufs=2))
        sm_pool = ctx.enter_context(tc.tile_pool(name="sm", bufs=2))
        oa_pool = ctx.enter_context(tc.tile_pool(name="oa", bufs=2))
        of_pool = ctx.enter_context(tc.tile_pool(name="of", bufs=2))

        state = {}

        def stage_a(it):
            int_t = int_pool.tile([P, N], f32, name="int_t", tag="int")
            nc.sync.dma_start(out=int_t[:], in_=INT[it * P : (it + 1) * P, :])
            E = e_pool.tile([P, N], f32, name="E", tag="E")
            racc = sm_pool.tile([P, NCH], f32, name="racc", tag="racc")
            for jc in range(NCH):
                ps = ps_pool.tile([P, CH], f32, name="ps", tag="ps")
                for et in range(DT):
                    nc.tensor.matmul(
                        ps[:],
                        qT[et][:, it * P : (it + 1) * P],
                        kT[et][:, jc * CH : (jc + 1) * CH],
                        start=(et == 0),
                        stop=(et == DT - 1),
                    )
                nc.scalar.activation(
                    E[:, jc * CH : (jc + 1) * CH],
                    ps[:],
                    Act.Exp,
                    bias=0.0,
                    scale=SCALE,
                    accum_out=racc[:, jc : jc + 1],
                )
            state[it] = (int_t, E, racc)

        def stage_b(it):
            int_t, E, racc = state.pop(it)
            r = sm_pool.tile([P, 1], f32, name="r", tag="r")
            nc.vector.reduce_sum(out=r[:], in_=racc[:], axis=mybir.AxisListType.X)
            rinv = sm_pool.tile([P, 1], f32, name="rinv", tag="rinv")
            nc.vector.reciprocal(rinv[:], r[:])

            # attn = E * (1/rowsum) + intensity, fused, chunked so the first
            # transposes can start before the whole row is normalized.
            for jc in range(NCH):
                sl = slice(jc * CH, (jc + 1) * CH)
                nc.vector.scalar_tensor_tensor(
                    out=E[:, sl],
                    in0=E[:, sl],
                    scalar=rinv[:],
                    in1=int_t[:, sl],
                    op0=Alu.mult,
                    op1=Alu.add,
                )

            # Transpose attn row-block: 4 PE transposes per PSUM bank, one
            # [128, 512] copy per bank.
            atc = []
            for g in range(NCH):
                pt = tp_pool.tile([P, D], f32, name="tp4", tag="tp4")
                for t in range(4):
                    jt = 4 * g + t
                    nc.tensor.transpose(
                        pt[:, t * P : (t + 1) * P],
                        E[:, jt * P : (jt + 1) * P],
                        ident[:],
                    )
                ac = at_pool.tile([P, D], mdt, name=f"atc{g}", tag=f"atc{g}")
                cp = nc.scalar.copy if g % 2 else nc.vector.tensor_copy
                cp(ac[:], pt[:])
                atc.append(ac)

            # out_attn[i, e] = sum_j attn^T[j, i].T @ v[j, e]
            ps2 = av_pool.tile([P, D], f32, name="ps2", tag="av")
            for jt in range(NT):
                nc.tensor.matmul(
                    ps2[:],
                    atc[jt // 4][:, (jt % 4) * P : (jt % 4 + 1) * P],
                    vt[jt][:],
                    start=(jt == 0),
                    stop=(jt == NT - 1),
                )
            oa = oa_pool.tile([P, D], f32, name="oa", tag="oa")
            nc.scalar.copy(oa[:], ps2[:])

            # Transpose out_attn block for the final projection (one bank).
            pt = tp_pool.tile([P, D], f32, name="tp4", tag="tp4")
            for et in range(DT):
                nc.tensor.transpose(
                    pt[:, et * P : (et + 1) * P],
                    oa[:, et * P : (et + 1) * P],
                    ident[:],
                )
            oatc = oa_pool.tile([P, D], mdt, name="oatc", tag="oatc")
            nc.vector.tensor_copy(oatc[:], pt[:])

            # out = out_attn @ OW^T + b, in natural [n, e] layout.
            psf = ps_pool.tile([P, D], f32, name="psf", tag="ps")
            for et in range(DT):
                nc.tensor.matmul(
                    psf[:],
                    oatc[:, et * P : (et + 1) * P],
                    owT[et][:],
                    start=(et == 0),
                    stop=(et == DT - 1),
                )
            of = of_pool.tile([P, D], f32, name="of", tag="of")
            nc.vector.scalar_tensor_tensor(
                out=of[:],
                in0=psf[:],
                scalar=0.0,
                in1=ob_bc[:],
                op0=Alu.bypass,
                op1=Alu.add,
            )
            nc.sync.dma_start(out=OUT[it * P : (it + 1) * P, :], in_=of[:])

        for it in range(NT + 1):
            if it < NT:
                stage_a(it)
            if it >= 1:
                stage_b(it - 1)


def build():
    import concourse.tile as tile
    from concourse import bacc, mybir

    f32 = mybir.dt.float32

    nc = bacc.Bacc("TRN2", target_bir_lowering=False, debug=False, num_devices=B)

    X = nc.dram_tensor("X", [N, D], f32, kind="ExternalInput").ap()
    INT = nc.dram_tensor("intensity", [N, N], f32, kind="ExternalInput").ap()
    WQ = nc.dram_tensor("WQ_w", [D, D], f32, kind="ExternalInput").ap()
    WQb = nc.dram_tensor("WQ_b", [D], f32, kind="ExternalInput").ap()
    WK = nc.dram_tensor("WK_w", [D, D], f32, kind="ExternalInput").ap()
    WKb = nc.dram_tensor("WK_b", [D], f32, kind="ExternalInput").ap()
    WV = nc.dram_tensor("WV_w", [D, D], f32, kind="ExternalInput").ap()
    WVb = nc.dram_tensor("WV_b", [D], f32, kind="ExternalInput").ap()
    OW = nc.dram_tensor("out_w", [D, D], f32, kind="ExternalInput").ap()
    OWb = nc.dram_tensor("out_b", [D], f32, kind="ExternalInput").ap()
    OUT = nc.dram_tensor("out", [N, D], f32, kind="ExternalOutput").ap()

    aps = (X, INT, WQ, WQb, WK, WKb, WV, WVb, OW, OWb, OUT)
    with tile.TileContext(nc) as tc:
        _emit(nc, tc, aps)
    nc.compile()
    return nc


def get_nc():
    if "nc" not in _CACHE:
        _CACHE["nc"] = build()
    return _CACHE["nc"]


def make_in_maps(**inputs):
    X = np.asarray(inputs["X"], dtype=np.float32)
    INT = np.asarray(inputs["intensity"], dtype=np.float32)
    shared = {
        name: np.asarray(inputs[name], dtype=np.float32)
        for name in ("WQ_w", "WQ_b", "WK_w", "WK_b", "WV_w", "WV_b", "out_w", "out_b")
    }
    return [{"X": X[b], "intensity": INT[b], **shared} for b in range(B)]


class SpmdRunner:
    """Cached PJRT executable for the SPMD program: compile once, run many.

    Mirrors concourse.bass2jax.run_bass_via_pjrt's multi-core path but keeps
    the jitted callable so repeated runs skip retracing/XLA recompilation,
    and inputs can stay resident on the devices.
    """

    def __init__(self, nc, n_cores=B):
        import jax
        from concourse import bass2jax, mybir
        from jax.experimental.shard_map import shard_map
        from jax.sharding import Mesh, NamedSharding, PartitionSpec

        bass2jax.install_neuronx_cc_hook()
        assert nc.dbg_addr is None
        partition_name = (
            nc.partition_id_tensor.name if nc.partition_id_tensor is not None else None
        )

        in_names, out_names, out_avals = [], [], []
        for alloc in nc.m.functions[0].allocations:
            if not isinstance(alloc, mybir.MemoryLocationSet):
                continue
            name = alloc.memorylocations[0].name
            if alloc.kind == "ExternalInput":
                if name != partition_name:
                    in_names.append(name)
            elif alloc.kind == "ExternalOutput":
                out_names.append(name)
                out_avals.append(
                    jax.core.ShapedArray(
                        tuple(alloc.tensor_shape), mybir.dt.np(alloc.dtype)
                    )
                )
        self.in_names, self.out_names, self.out_avals = in_names, out_names, out_avals
        self.n_cores = n_cores
        n_params, n_outs = len(in_names), len(out_names)
        all_in_names = list(in_names) + list(out_names)
        if partition_name is not None:
            all_in_names.append(partition_name)
        all_in_names = tuple(all_in_names)

        def _body(*args):
            operands = list(args)
            if partition_name is not None:
                operands.append(bass2jax.partition_id_tensor())
            outs = bass2jax._bass_exec_p.bind(
                *operands,
                out_avals=tuple(out_avals),
                in_names=all_in_names,
                out_names=tuple(out_names),
                lowering_input_output_aliases=(),
                sim_require_finite=True,
                sim_require_nnan=True,
                nc=nc,
            )
            return tuple(outs)

        devices = jax.devices()[:n_cores]
        self.mesh = Mesh(np.asarray(devices), ("core",))
        spec = PartitionSpec("core")
        self.sharding = NamedSharding(self.mesh, spec)
        donate = tuple(range(n_params, n_params + n_outs))
        self._fn = jax.jit(
            shard_map(
                _body,
                mesh=self.mesh,
                in_specs=(spec,) * (n_params + n_outs),
                out_specs=(spec,) * n_outs,
                check_rep=False,
            ),
            donate_argnums=donate,
            keep_unused=True,
        )

    def stage_inputs(self, in_maps):
        import jax

        concat = [
            np.concatenate(
                [np.asarray(in_maps[c][n]) for c in range(self.n_cores)], axis=0
            )
            for n in self.in_names
        ]
        return [jax.device_put(a, self.sharding) for a in concat]

    def make_zeros(self):
        import jax
        import jax.numpy as jnp

        zs = []
        for av in self.out_avals:
            shape = (self.n_cores * av.shape[0], *av.shape[1:])
            z = jax.jit(
                lambda shape=shape, dtype=av.dtype: jnp.zeros(shape, dtype),
                out_shardings=self.sharding,
            )()
            zs.append(z)
        return zs

    def run(self, staged, zeros):
        outs = self._fn(*staged, *zeros)
        for o in outs:
            o.block_until_ready()
        return outs

    def gather(self, outs):
        per_out = []
        for i, av in enumerate(self.out_avals):
            arr = np.asarray(outs[i]).reshape(self.n_cores, *av.shape)
            per_out.append(arr)
        return dict(zip(self.out_names, per_out))


def get_runner():
    if "runner" not in _CACHE:
        _CACHE["runner"] = SpmdRunner(get_nc())
    return _CACHE["runner"]


def kernel(**inputs):
    runner = get_runner()
    in_maps = make_in_maps(**inputs)
    staged = runner.stage_inputs(in_maps)
    outs = runner.run(staged, runner.make_zeros())
    return runner.gather(outs)["out"].astype(np.float32)
